# revision 1
# baseline (speedup 1.0000x reference)
"""Multi-head attention (B=4, S=2048, D=1024, H=16) on 8 TRN2 NeuronCores.

Sharding: 2D grid batch(4) x head-group(2). Core c handles batch c//2 and
heads [ (c%2)*8 , (c%2)*8+8 ). Each core computes:
  - Q,K projections for its 8 heads in transposed layout q.T/k.T [512, 2048]
    (spilled to DRAM scratch, reloaded per 2-head group)
  - V projection in natural layout [2048, 512], stored ones-augmented per head
    ([128, 16, 8, 65] with col 64 = 1.0) so the AV matmul also produces the
    softmax denominators
  - attention per head: S.T = K Q^T (scores transposed), exp via ACT (scale
    fused), AV matmul accumulating C.T[hd,sq] + denom row, normalize via
    reciprocal + gpsimd partition-broadcast
  - output projection out.T = Wo_p.T^T @ ctx.T accumulated over head groups
Host side: shard/rearrange inputs, sum the two partial outputs per batch,
add bo, transpose back.

All matmuls run in float32r (fp32 storage, ~bf16 speed, ~1e-4 matmul error).
"""
import sys

sys.path.insert(0, "/opt/trn_rl_repo")

import types

# antenv.axon_hooks is missing from this image; install a shim so
# run_bass_kernel_spmd(trace=True) can reach the NTFF profiler.
if "antenv.axon_hooks" not in sys.modules:
    _mod = types.ModuleType("antenv.axon_hooks")
    _hook = [None]
    _mod.set_axon_ntff_profile_hook = lambda h: _hook.__setitem__(0, h)
    _mod.get_axon_ntff_profile_hook = lambda: _hook[0]
    sys.modules["antenv.axon_hooks"] = _mod
    import antenv

    antenv.axon_hooks = _mod
    try:
        from trn_agent_boot.trn_boot import _ntff_profile_via_ctypes

        _mod.set_axon_ntff_profile_hook(
            _ntff_profile_via_ctypes("/opt/axon/libaxon_pjrt.so")
        )
    except Exception:
        pass

import numpy as np
import concourse.bass as bass
import concourse.mybir as mybir
import concourse.tile as tile
from concourse import bacc
from concourse import bass_utils

B, S, D, H = 4, 2048, 1024, 16
HD = D // H  # 64
SCALE = HD ** -0.5
NCORES = 8
HPC = 8          # heads per core
JP = HPC * HD    # 512 projected feature cols per core
G = 4            # 2-head groups per core
DC = D // 128    # 8 contraction chunks
SBP = 256        # s-block for projections
NSB = S // SBP   # 8
NST = S // 128   # 16 s-tiles
F32 = mybir.dt.float32
F32R = mybir.dt.float32r

_CACHE = {}


def _build():
    nc = bacc.Bacc("TRN2", target_bir_lowering=False, debug=False)

    xq = nc.dram_tensor("xq", [NSB, 128, DC, SBP], F32R, kind="ExternalInput").ap()
    xk = nc.dram_tensor("xk", [NSB, 128, DC, SBP], F32R, kind="ExternalInput").ap()
    xv = nc.dram_tensor("xv", [NSB, 128, DC, SBP], F32R, kind="ExternalInput").ap()
    wq = nc.dram_tensor("wq", [128, DC, JP], F32R, kind="ExternalInput").ap()
    wk = nc.dram_tensor("wk", [128, DC, JP], F32R, kind="ExternalInput").ap()
    wv = nc.dram_tensor("wv", [128, DC, JP], F32R, kind="ExternalInput").ap()
    wo = nc.dram_tensor("wo", [128, G, D], F32R, kind="ExternalInput").ap()
    bq = nc.dram_tensor("bq", [128, 4], F32, kind="ExternalInput").ap()
    bk = nc.dram_tensor("bk", [128, 4], F32, kind="ExternalInput").ap()
    bv = nc.dram_tensor("bv", [1, JP], F32, kind="ExternalInput").ap()
    out = nc.dram_tensor("out", [D, S], F32, kind="ExternalOutput").ap()

    with tile.TileContext(nc) as tc:
        with (
            tc.tile_pool(name="consts", bufs=1) as consts,
            tc.tile_pool(name="wts", bufs=3) as wts,
            tc.tile_pool(name="xs", bufs=2) as xs,
            tc.tile_pool(name="stage", bufs=3) as stage,
            tc.tile_pool(name="vpool", bufs=1) as vpool,
            tc.tile_pool(name="qk", bufs=2) as qkp,
            tc.tile_pool(name="ex", bufs=2) as exp_pool,
            tc.tile_pool(name="nrm", bufs=2) as nrm,
            tc.tile_pool(name="ctxp", bufs=4) as ctxp,
            tc.tile_pool(name="outst", bufs=3) as outst,
            tc.tile_pool(name="scr", bufs=1, space="DRAM") as scr,
        ):
            qsc = scr.tile([G, 128, S], F32R, tag="qsc")
            ksc = scr.tile([G, 128, S], F32R, tag="ksc")

            bq_sb = consts.tile([128, 4], F32, tag="bq")
            nc.sync.dma_start(out=bq_sb[:], in_=bq[:])
            bk_sb = consts.tile([128, 4], F32, tag="bk")
            nc.sync.dma_start(out=bk_sb[:], in_=bk[:])
            bv_row = consts.tile([1, JP], F32, tag="bvr")
            nc.sync.dma_start(out=bv_row[:], in_=bv[:])
            bv_bc = consts.tile([128, JP], F32, tag="bvb")
            nc.gpsimd.partition_broadcast(bv_bc[:], bv_row[:])

            # V' resident tile: [p, s_tile, head, hd+1], col 64 = ones.
            v_sb = vpool.tile([128, NST, HPC, HD + 1], F32R, tag="v")
            nc.vector.memset(v_sb[:, :, :, HD:HD + 1].bitcast(F32), 1.0)

            # ---------------- Q/K projections -> DRAM scratch ----------------
            with tc.tile_pool(name="pps", bufs=1, space="PSUM") as pps:
                for pname, xdram, wdram, bias_sb, scratch in (
                    ("q", xq, wq, bq_sb, qsc),
                    ("k", xk, wk, bk_sb, ksc),
                ):
                    w_sb = wts.tile([128, DC, JP], F32R, tag="w", name=f"w{pname}_sb")
                    nc.sync.dma_start(out=w_sb[:], in_=wdram[:])
                    for sb in range(NSB):
                        x_sb = xs.tile([128, DC, SBP], F32R, tag="x",
                                       name=f"x{pname}_{sb}")
                        nc.sync.dma_start(out=x_sb[:], in_=xdram[sb])
                        for jt in range(4):
                            ps_t = pps.tile([128, SBP], F32, tag="qk", bufs=4,
                                            name=f"ps{pname}_{sb}_{jt}")
                            for dc in range(DC):
                                nc.tensor.matmul(
                                    ps_t[:],
                                    w_sb[:, dc, jt * 128:(jt + 1) * 128],
                                    x_sb[:, dc, :],
                                    start=(dc == 0), stop=(dc == DC - 1),
                                )
                            st_t = stage.tile([128, SBP], F32R, tag="stg",
                                              name=f"st{pname}_{sb}_{jt}")
                            nc.vector.tensor_scalar_add(
                                out=st_t[:], in0=ps_t[:],
                                scalar1=bias_sb[:, jt:jt + 1],
                            )
                            nc.sync.dma_start(
                                out=scratch[jt, :, sb * SBP:(sb + 1) * SBP],
                                in_=st_t[:],
                            )

                # ---------------- V projection -> resident v_sb ----------------
                wv_sb = wts.tile([128, DC, JP], F32R, tag="w", name="wv_sb")
                nc.sync.dma_start(out=wv_sb[:], in_=wv[:])
                for sb in range(NSB):
                    xv_sb = xs.tile([128, DC, SBP], F32R, tag="x", name=f"xv_{sb}")
                    nc.sync.dma_start(out=xv_sb[:], in_=xv[sb])
                    for half in range(2):
                        sti = sb * 2 + half
                        ps_v = pps.tile([128, JP], F32, tag="v", bufs=2,
                                        name=f"psv_{sti}")
                        for dc in range(DC):
                            nc.tensor.matmul(
                                ps_v[:],
                                xv_sb[:, dc, half * 128:(half + 1) * 128],
                                wv_sb[:, dc, :],
                                start=(dc == 0), stop=(dc == DC - 1),
                            )
                        nc.vector.tensor_add(
                            out=v_sb[:, sti, :, 0:HD],
                            in0=ps_v.rearrange("p (h d) -> p h d", h=HPC),
                            in1=bv_bc.rearrange("p (h d) -> p h d", h=HPC),
                        )

            # ---------------- attention ----------------
            ctx_tiles = []
            with tc.tile_pool(name="aps", bufs=1, space="PSUM") as aps:
                for g in range(G):
                    q_sb = qkp.tile([128, S], F32R, tag="qg", name=f"qg_{g}")
                    nc.sync.dma_start(out=q_sb[:], in_=qsc[g])
                    k_sb = qkp.tile([128, S], F32R, tag="kg", name=f"kg_{g}")
                    nc.sync.dma_start(out=k_sb[:], in_=ksc[g])
                    ctx_t = ctxp.tile([128, S], F32R, tag="ctx", name=f"ctx_{g}")
                    ctx_tiles.append(ctx_t)
                    for hh in range(2):
                        p0 = hh * HD
                        hloc = 2 * g + hh
                        for sqp in range(2):
                            q0 = sqp * 1024
                            av = aps.tile([HD + 1, 2, 512], F32, tag="av", bufs=2,
                                          name=f"av_{g}_{hh}_{sqp}")
                            for sk in range(NST):
                                st = aps.tile([128, 2, 512], F32, tag="st", bufs=2,
                                              name=f"stt_{g}_{hh}_{sqp}_{sk}")
                                for i in range(2):
                                    nc.tensor.matmul(
                                        st[:, i, :],
                                        k_sb[p0:p0 + HD, sk * 128:(sk + 1) * 128],
                                        q_sb[p0:p0 + HD, q0 + i * 512:q0 + (i + 1) * 512],
                                        start=True, stop=True,
                                    )
                                ex_t = exp_pool.tile([128, 2, 512], F32R, tag="ex",
                                                     name=f"ex_{g}_{hh}_{sqp}_{sk}")
                                nc.scalar.activation(
                                    out=ex_t[:], in_=st[:],
                                    func=mybir.ActivationFunctionType.Exp,
                                    scale=SCALE,
                                )
                                for i in range(2):
                                    nc.tensor.matmul(
                                        av[:, i, :],
                                        v_sb[:, sk, hloc, :],
                                        ex_t[:, i, :],
                                        start=(sk == 0), stop=(sk == NST - 1),
                                    )
                            for i in range(2):
                                recip = nrm.tile([1, 512], F32, tag="rc",
                                                 name=f"rc_{g}_{hh}_{sqp}_{i}")
                                nc.vector.reciprocal(recip[:], av[HD:HD + 1, i, :])
                                bcast = nrm.tile([HD, 512], F32, tag="bc",
                                                 name=f"bc_{g}_{hh}_{sqp}_{i}")
                                nc.gpsimd.partition_broadcast(bcast[:], recip[:])
                                nc.vector.tensor_mul(
                                    out=ctx_t[p0:p0 + HD,
                                              q0 + i * 512:q0 + (i + 1) * 512],
                                    in0=av[0:HD, i, :],
                                    in1=bcast[:],
                                )

            # ---------------- output projection ----------------
            with tc.tile_pool(name="ops", bufs=1, space="PSUM") as ops:
                wo_sb = wts.tile([128, G, D], F32R, tag="w", name="wo_sb")
                nc.sync.dma_start(out=wo_sb[:], in_=wo[:])
                for et in range(D // 128):
                    for sb4 in range(4):
                        ps_o = ops.tile([128, 512], F32, tag="o", bufs=4,
                                        name=f"pso_{et}_{sb4}")
                        for g in range(G):
                            nc.tensor.matmul(
                                ps_o[:],
                                wo_sb[:, g, et * 128:(et + 1) * 128],
                                ctx_tiles[g][:, sb4 * 512:(sb4 + 1) * 512],
                                start=(g == 0), stop=(g == G - 1),
                            )
                        o_sb = outst.tile([128, 512], F32, tag="os",
                                          name=f"os_{et}_{sb4}")
                        nc.vector.tensor_copy(out=o_sb[:], in_=ps_o[:])
                        nc.sync.dma_start(
                            out=out[et * 128:(et + 1) * 128,
                                    sb4 * 512:(sb4 + 1) * 512],
                            in_=o_sb[:],
                        )
    nc.compile()
    return nc


def get_nc():
    if "nc" not in _CACHE:
        _CACHE["nc"] = _build()
    return _CACHE["nc"]


def _arrange_x(x):
    """[S, D] -> [NSB, 128, DC, SBP] with [sb, p, dc, s] = x[sb*SBP+s, dc*128+p]."""
    return np.ascontiguousarray(
        x.reshape(NSB, SBP, DC, 128).transpose(0, 3, 2, 1)
    )


def _arrange_w(w_p):
    """W_p [JP, D] (rows for this core's heads) -> [128, DC, JP] lhsT layout."""
    # want [p, dc, j] = W_p.T[dc*128+p, j] = W_p[j, dc*128+p]
    return np.ascontiguousarray(w_p.T.reshape(DC, 128, JP).transpose(1, 0, 2))


def _arrange_wo(wo_p):
    """Wo_p = Wo[:, cols] [D, JP] -> [128, G, D] with [p,g,e] = Wo_p[e, g*128+p]."""
    return np.ascontiguousarray(wo_p.T.reshape(G, 128, D).transpose(1, 0, 2))


def prepare_in_maps(query, key, value, Wq, bq, Wk, bk, Wv, bv, Wo, bo):
    xs_arr = {}
    for b in range(B):
        xs_arr[("q", b)] = _arrange_x(np.asarray(query[b], np.float32))
        xs_arr[("k", b)] = _arrange_x(np.asarray(key[b], np.float32))
        xs_arr[("v", b)] = _arrange_x(np.asarray(value[b], np.float32))
    ws = {}
    for gidx in range(2):
        rows = slice(gidx * JP, (gidx + 1) * JP)
        ws[("wq", gidx)] = _arrange_w(np.asarray(Wq, np.float32)[rows])
        ws[("wk", gidx)] = _arrange_w(np.asarray(Wk, np.float32)[rows])
        ws[("wv", gidx)] = _arrange_w(np.asarray(Wv, np.float32)[rows])
        ws[("wo", gidx)] = _arrange_wo(np.asarray(Wo, np.float32)[:, rows])
        ws[("bq", gidx)] = np.ascontiguousarray(
            np.asarray(bq, np.float32)[rows].reshape(4, 128).T)
        ws[("bk", gidx)] = np.ascontiguousarray(
            np.asarray(bk, np.float32)[rows].reshape(4, 128).T)
        ws[("bv", gidx)] = np.asarray(bv, np.float32)[rows].reshape(1, JP).copy()
    in_maps = []
    for c in range(NCORES):
        b, gidx = c // 2, c % 2
        in_maps.append({
            "xq": xs_arr[("q", b)],
            "xk": xs_arr[("k", b)],
            "xv": xs_arr[("v", b)],
            "wq": ws[("wq", gidx)],
            "wk": ws[("wk", gidx)],
            "wv": ws[("wv", gidx)],
            "wo": ws[("wo", gidx)],
            "bq": ws[("bq", gidx)],
            "bk": ws[("bk", gidx)],
            "bv": ws[("bv", gidx)],
        })
    return in_maps


def run_hw(inputs, trace=False, trace_cores=None):
    nc = get_nc()
    in_maps = prepare_in_maps(**inputs)
    res = bass_utils.run_bass_kernel_spmd(
        nc, in_maps, core_ids=list(range(NCORES)),
        trace=trace, trace_cores=trace_cores,
    )
    bo = np.asarray(inputs["bo"], np.float32)
    out = np.empty((B, S, D), np.float32)
    for b in range(B):
        acc = res.results[2 * b]["out"] + res.results[2 * b + 1]["out"]
        out[b] = acc.T + bo
    return out, res


def kernel(**inputs):
    out, _ = run_hw(inputs, trace=False)
    return out


if __name__ == "__main__":
    rng = np.random.default_rng(0)
    ins = {
        "query": rng.standard_normal((B, S, D), np.float32),
        "key": rng.standard_normal((B, S, D), np.float32),
        "value": rng.standard_normal((B, S, D), np.float32),
        "Wq": (rng.standard_normal((D, D)) * D ** -0.5).astype(np.float32),
        "bq": np.zeros(D, np.float32),
        "Wk": (rng.standard_normal((D, D)) * D ** -0.5).astype(np.float32),
        "bk": np.zeros(D, np.float32),
        "Wv": (rng.standard_normal((D, D)) * D ** -0.5).astype(np.float32),
        "bv": np.zeros(D, np.float32),
        "Wo": (rng.standard_normal((D, D)) * D ** -0.5).astype(np.float32),
        "bo": np.zeros(D, np.float32),
    }
    out = kernel(**ins)
    print("kernel out", out.shape, out.dtype, float(np.abs(out).mean()))


# revision 4
# speedup vs baseline: 1.0210x; 1.0210x over previous
"""Multi-head attention (B=4, S=2048, D=1024, H=16) on 8 TRN2 NeuronCores.

Sharding: 2D grid batch(4) x head-group(2). Core c handles batch c//2 and
heads [ (c%2)*8 , (c%2)*8+8 ). Each core computes:
  - Q,K projections for its 8 heads in transposed layout q.T/k.T [512, 2048]
    (spilled to DRAM scratch, reloaded per 2-head group)
  - V projection in natural layout [2048, 512], stored ones-augmented per head
    ([128, 16, 8, 65] with col 64 = 1.0) so the AV matmul also produces the
    softmax denominators
  - attention per head: S.T = K Q^T (scores transposed), exp via ACT (scale
    fused), AV matmul accumulating C.T[hd,sq] + denom row, normalize via
    reciprocal + gpsimd partition-broadcast
  - output projection out.T = Wo_p.T^T @ ctx.T accumulated over head groups
Host side: shard/rearrange inputs, sum the two partial outputs per batch,
add bo, transpose back.

Matmul dtype: bfloat16 by default (KERNEL_DTYPE=f32r for float32r fallback).
PSUM accumulation is always fp32; biases and normalization are fp32.
"""
import os
import sys

sys.path.insert(0, "/opt/trn_rl_repo")

import types

# antenv.axon_hooks is missing from this image; install a shim so
# run_bass_kernel_spmd(trace=True) can reach the NTFF profiler.
if "antenv.axon_hooks" not in sys.modules:
    _mod = types.ModuleType("antenv.axon_hooks")
    _hook = [None]
    _mod.set_axon_ntff_profile_hook = lambda h: _hook.__setitem__(0, h)
    _mod.get_axon_ntff_profile_hook = lambda: _hook[0]
    sys.modules["antenv.axon_hooks"] = _mod
    import antenv

    antenv.axon_hooks = _mod
    try:
        from trn_agent_boot.trn_boot import _ntff_profile_via_ctypes

        _mod.set_axon_ntff_profile_hook(
            _ntff_profile_via_ctypes("/opt/axon/libaxon_pjrt.so")
        )
    except Exception:
        pass

import ml_dtypes
import numpy as np
import concourse.bass as bass
import concourse.mybir as mybir
import concourse.tile as tile
from concourse import bacc
from concourse import bass_utils

B, S, D, H = 4, 2048, 1024, 16
HD = D // H  # 64
SCALE = HD ** -0.5
NCORES = 8
HPC = 8          # heads per core
JP = HPC * HD    # 512 projected feature cols per core
G = 4            # 2-head groups per core
DC = D // 128    # 8 contraction chunks
NST = S // 128   # 16 s-tiles
F32 = mybir.dt.float32
F32R = mybir.dt.float32r

DTYPE_NAME = os.environ.get("KERNEL_DTYPE", "bf16")
if DTYPE_NAME == "bf16":
    MDT = mybir.dt.bfloat16
    NPDT = ml_dtypes.bfloat16
    SBP = 512    # s-block for projections
else:
    MDT = F32R
    NPDT = np.float32
    SBP = 256
NSB = S // SBP

_CACHE = {}


def _build():
    nc = bacc.Bacc("TRN2", target_bir_lowering=False, debug=False)

    xq = nc.dram_tensor("xq", [NSB, 128, DC, SBP], MDT, kind="ExternalInput").ap()
    xk = nc.dram_tensor("xk", [NSB, 128, DC, SBP], MDT, kind="ExternalInput").ap()
    xv = nc.dram_tensor("xv", [NSB, 128, DC, SBP], MDT, kind="ExternalInput").ap()
    wq = nc.dram_tensor("wq", [128, DC, JP], MDT, kind="ExternalInput").ap()
    wk = nc.dram_tensor("wk", [128, DC, JP], MDT, kind="ExternalInput").ap()
    wv = nc.dram_tensor("wv", [128, DC, JP], MDT, kind="ExternalInput").ap()
    wo = nc.dram_tensor("wo", [128, G, D], MDT, kind="ExternalInput").ap()
    bq = nc.dram_tensor("bq", [128, 4], F32, kind="ExternalInput").ap()
    bk = nc.dram_tensor("bk", [128, 4], F32, kind="ExternalInput").ap()
    bv = nc.dram_tensor("bv", [1, JP], F32, kind="ExternalInput").ap()
    out = nc.dram_tensor("out", [D, S], F32, kind="ExternalOutput").ap()

    with tile.TileContext(nc) as tc:
        with (
            tc.tile_pool(name="consts", bufs=1) as consts,
            tc.tile_pool(name="wts", bufs=3) as wts,
            tc.tile_pool(name="xs", bufs=2) as xs,
            tc.tile_pool(name="stage", bufs=3) as stage,
            tc.tile_pool(name="vpool", bufs=1) as vpool,
            tc.tile_pool(name="qk", bufs=2) as qkp,
            tc.tile_pool(name="ex", bufs=2) as exp_pool,
            tc.tile_pool(name="nrm", bufs=2) as nrm,
            tc.tile_pool(name="ctxp", bufs=4) as ctxp,
            tc.tile_pool(name="outst", bufs=3) as outst,
            tc.tile_pool(name="scr", bufs=1, space="DRAM") as scr,
        ):
            qsc = scr.tile([G, 128, S], MDT, tag="qsc")
            ksc = scr.tile([G, 128, S], MDT, tag="ksc")

            bq_sb = consts.tile([128, 4], F32, tag="bq")
            nc.sync.dma_start(out=bq_sb[:], in_=bq[:])
            bk_sb = consts.tile([128, 4], F32, tag="bk")
            nc.sync.dma_start(out=bk_sb[:], in_=bk[:])
            bv_row = consts.tile([1, JP], F32, tag="bvr")
            nc.sync.dma_start(out=bv_row[:], in_=bv[:])
            bv_bc = consts.tile([128, JP], F32, tag="bvb")
            nc.gpsimd.partition_broadcast(bv_bc[:], bv_row[:])

            # V' resident tile: [p, s_tile, head, hd+1], col 64 = ones.
            v_sb = vpool.tile([128, NST, HPC, HD + 1], MDT, tag="v")
            if MDT == F32R:
                nc.vector.memset(v_sb[:, :, :, HD:HD + 1].bitcast(F32), 1.0)
            else:
                nc.vector.memset(v_sb[:, :, :, HD:HD + 1], 1.0)

            # ---------------- Q/K projections -> DRAM scratch ----------------
            with tc.tile_pool(name="pps", bufs=1, space="PSUM") as pps:
                for pname, xdram, wdram, bias_sb, scratch in (
                    ("q", xq, wq, bq_sb, qsc),
                    ("k", xk, wk, bk_sb, ksc),
                ):
                    w_sb = wts.tile([128, DC, JP], MDT, tag="w", name=f"w{pname}_sb")
                    nc.sync.dma_start(out=w_sb[:], in_=wdram[:])
                    for sb in range(NSB):
                        x_sb = xs.tile([128, DC, SBP], MDT, tag="x",
                                       name=f"x{pname}_{sb}")
                        nc.sync.dma_start(out=x_sb[:], in_=xdram[sb])
                        for jt in range(4):
                            ps_t = pps.tile([128, SBP], F32, tag="qk", bufs=4,
                                            name=f"ps{pname}_{sb}_{jt}")
                            for dc in range(DC):
                                nc.tensor.matmul(
                                    ps_t[:],
                                    w_sb[:, dc, jt * 128:(jt + 1) * 128],
                                    x_sb[:, dc, :],
                                    start=(dc == 0), stop=(dc == DC - 1),
                                )
                            st_t = stage.tile([128, SBP], MDT, tag="stg",
                                              name=f"st{pname}_{sb}_{jt}")
                            nc.vector.tensor_scalar_add(
                                out=st_t[:], in0=ps_t[:],
                                scalar1=bias_sb[:, jt:jt + 1],
                            )
                            nc.sync.dma_start(
                                out=scratch[jt, :, sb * SBP:(sb + 1) * SBP],
                                in_=st_t[:],
                            )

                # ---------------- V projection -> resident v_sb ----------------
                wv_sb = wts.tile([128, DC, JP], MDT, tag="w", name="wv_sb")
                nc.sync.dma_start(out=wv_sb[:], in_=wv[:])
                for sb in range(NSB):
                    xv_sb = xs.tile([128, DC, SBP], MDT, tag="x", name=f"xv_{sb}")
                    nc.sync.dma_start(out=xv_sb[:], in_=xv[sb])
                    for half in range(SBP // 128):
                        sti = sb * (SBP // 128) + half
                        ps_v = pps.tile([128, JP], F32, tag="v", bufs=2,
                                        name=f"psv_{sti}")
                        for dc in range(DC):
                            nc.tensor.matmul(
                                ps_v[:],
                                xv_sb[:, dc, half * 128:(half + 1) * 128],
                                wv_sb[:, dc, :],
                                start=(dc == 0), stop=(dc == DC - 1),
                            )
                        nc.vector.tensor_add(
                            out=v_sb[:, sti, :, 0:HD],
                            in0=ps_v.rearrange("p (h d) -> p h d", h=HPC),
                            in1=bv_bc.rearrange("p (h d) -> p h d", h=HPC),
                        )

            # ---------------- attention ----------------
            ctx_tiles = []
            with tc.tile_pool(name="aps", bufs=1, space="PSUM") as aps:
                for g in range(G):
                    q_sb = qkp.tile([128, S], MDT, tag="qg", name=f"qg_{g}")
                    nc.sync.dma_start(out=q_sb[:], in_=qsc[g])
                    k_sb = qkp.tile([128, S], MDT, tag="kg", name=f"kg_{g}")
                    nc.sync.dma_start(out=k_sb[:], in_=ksc[g])
                    ctx_t = ctxp.tile([128, S], MDT, tag="ctx", name=f"ctx_{g}")
                    ctx_tiles.append(ctx_t)
                    for hh in range(2):
                        p0 = hh * HD
                        hloc = 2 * g + hh
                        for sqp in range(2):
                            q0 = sqp * 1024
                            av = aps.tile([HD + 1, 2, 512], F32, tag="av", bufs=2,
                                          name=f"av_{g}_{hh}_{sqp}")
                            for sk in range(NST):
                                st = aps.tile([128, 2, 512], F32, tag="st", bufs=2,
                                              name=f"stt_{g}_{hh}_{sqp}_{sk}")
                                for i in range(2):
                                    nc.tensor.matmul(
                                        st[:, i, :],
                                        k_sb[p0:p0 + HD, sk * 128:(sk + 1) * 128],
                                        q_sb[p0:p0 + HD, q0 + i * 512:q0 + (i + 1) * 512],
                                        start=True, stop=True,
                                    )
                                ex_t = exp_pool.tile([128, 2, 512], MDT, tag="ex",
                                                     name=f"ex_{g}_{hh}_{sqp}_{sk}")
                                nc.scalar.activation(
                                    out=ex_t[:], in_=st[:],
                                    func=mybir.ActivationFunctionType.Exp,
                                    scale=SCALE,
                                )
                                for i in range(2):
                                    nc.tensor.matmul(
                                        av[:, i, :],
                                        v_sb[:, sk, hloc, :],
                                        ex_t[:, i, :],
                                        start=(sk == 0), stop=(sk == NST - 1),
                                    )
                            for i in range(2):
                                dcp = nrm.tile([1, 512], F32, tag="dcp",
                                               name=f"dcp_{g}_{hh}_{sqp}_{i}")
                                nc.vector.tensor_copy(
                                    out=dcp[:], in_=av[HD:HD + 1, i, :])
                                recip = nrm.tile([1, 512], F32, tag="rc",
                                                 name=f"rc_{g}_{hh}_{sqp}_{i}")
                                nc.vector.reciprocal_approx_fast(
                                    recip[:], dcp[:])
                                bcast = nrm.tile([HD, 512], F32, tag="bc",
                                                 name=f"bc_{g}_{hh}_{sqp}_{i}")
                                nc.gpsimd.partition_broadcast(bcast[:], recip[:])
                                nc.vector.tensor_mul(
                                    out=ctx_t[p0:p0 + HD,
                                              q0 + i * 512:q0 + (i + 1) * 512],
                                    in0=av[0:HD, i, :],
                                    in1=bcast[:],
                                )

            # ---------------- output projection ----------------
            with tc.tile_pool(name="ops", bufs=1, space="PSUM") as ops:
                wo_sb = wts.tile([128, G, D], MDT, tag="w", name="wo_sb")
                nc.sync.dma_start(out=wo_sb[:], in_=wo[:])
                for et in range(D // 128):
                    for sb4 in range(4):
                        ps_o = ops.tile([128, 512], F32, tag="o", bufs=4,
                                        name=f"pso_{et}_{sb4}")
                        for g in range(G):
                            nc.tensor.matmul(
                                ps_o[:],
                                wo_sb[:, g, et * 128:(et + 1) * 128],
                                ctx_tiles[g][:, sb4 * 512:(sb4 + 1) * 512],
                                start=(g == 0), stop=(g == G - 1),
                            )
                        o_sb = outst.tile([128, 512], F32, tag="os",
                                          name=f"os_{et}_{sb4}")
                        nc.vector.tensor_copy(out=o_sb[:], in_=ps_o[:])
                        nc.sync.dma_start(
                            out=out[et * 128:(et + 1) * 128,
                                    sb4 * 512:(sb4 + 1) * 512],
                            in_=o_sb[:],
                        )
    nc.compile()
    return nc


def get_nc():
    if "nc" not in _CACHE:
        _CACHE["nc"] = _build()
    return _CACHE["nc"]


def _arrange_x(x):
    """[S, D] -> [NSB, 128, DC, SBP] with [sb, p, dc, s] = x[sb*SBP+s, dc*128+p]."""
    return np.ascontiguousarray(
        x.reshape(NSB, SBP, DC, 128).transpose(0, 3, 2, 1)
    ).astype(NPDT)


def _arrange_w(w_p):
    """W_p [JP, D] (rows for this core's heads) -> [128, DC, JP] lhsT layout."""
    # want [p, dc, j] = W_p.T[dc*128+p, j] = W_p[j, dc*128+p]
    return np.ascontiguousarray(
        w_p.T.reshape(DC, 128, JP).transpose(1, 0, 2)).astype(NPDT)


def _arrange_wo(wo_p):
    """Wo_p = Wo[:, cols] [D, JP] -> [128, G, D] with [p,g,e] = Wo_p[e, g*128+p]."""
    return np.ascontiguousarray(
        wo_p.T.reshape(G, 128, D).transpose(1, 0, 2)).astype(NPDT)


def prepare_in_maps(query, key, value, Wq, bq, Wk, bk, Wv, bv, Wo, bo):
    xs_arr = {}
    for b in range(B):
        xs_arr[("q", b)] = _arrange_x(np.asarray(query[b], np.float32))
        xs_arr[("k", b)] = _arrange_x(np.asarray(key[b], np.float32))
        xs_arr[("v", b)] = _arrange_x(np.asarray(value[b], np.float32))
    ws = {}
    for gidx in range(2):
        rows = slice(gidx * JP, (gidx + 1) * JP)
        ws[("wq", gidx)] = _arrange_w(np.asarray(Wq, np.float32)[rows])
        ws[("wk", gidx)] = _arrange_w(np.asarray(Wk, np.float32)[rows])
        ws[("wv", gidx)] = _arrange_w(np.asarray(Wv, np.float32)[rows])
        ws[("wo", gidx)] = _arrange_wo(np.asarray(Wo, np.float32)[:, rows])
        ws[("bq", gidx)] = np.ascontiguousarray(
            np.asarray(bq, np.float32)[rows].reshape(4, 128).T)
        ws[("bk", gidx)] = np.ascontiguousarray(
            np.asarray(bk, np.float32)[rows].reshape(4, 128).T)
        ws[("bv", gidx)] = np.asarray(bv, np.float32)[rows].reshape(1, JP).copy()
    in_maps = []
    for c in range(NCORES):
        b, gidx = c // 2, c % 2
        in_maps.append({
            "xq": xs_arr[("q", b)],
            "xk": xs_arr[("k", b)],
            "xv": xs_arr[("v", b)],
            "wq": ws[("wq", gidx)],
            "wk": ws[("wk", gidx)],
            "wv": ws[("wv", gidx)],
            "wo": ws[("wo", gidx)],
            "bq": ws[("bq", gidx)],
            "bk": ws[("bk", gidx)],
            "bv": ws[("bv", gidx)],
        })
    return in_maps


def run_hw(inputs, trace=False, trace_cores=None):
    nc = get_nc()
    in_maps = prepare_in_maps(**inputs)
    res = bass_utils.run_bass_kernel_spmd(
        nc, in_maps, core_ids=list(range(NCORES)),
        trace=trace, trace_cores=trace_cores,
    )
    bo = np.asarray(inputs["bo"], np.float32)
    out = np.empty((B, S, D), np.float32)
    for b in range(B):
        acc = res.results[2 * b]["out"] + res.results[2 * b + 1]["out"]
        out[b] = acc.T + bo
    return out, res


def kernel(**inputs):
    out, _ = run_hw(inputs, trace=False)
    return out


if __name__ == "__main__":
    rng = np.random.default_rng(0)
    ins = {
        "query": rng.standard_normal((B, S, D), np.float32),
        "key": rng.standard_normal((B, S, D), np.float32),
        "value": rng.standard_normal((B, S, D), np.float32),
        "Wq": (rng.standard_normal((D, D)) * D ** -0.5).astype(np.float32),
        "bq": np.zeros(D, np.float32),
        "Wk": (rng.standard_normal((D, D)) * D ** -0.5).astype(np.float32),
        "bk": np.zeros(D, np.float32),
        "Wv": (rng.standard_normal((D, D)) * D ** -0.5).astype(np.float32),
        "bv": np.zeros(D, np.float32),
        "Wo": (rng.standard_normal((D, D)) * D ** -0.5).astype(np.float32),
        "bo": np.zeros(D, np.float32),
    }
    out = kernel(**ins)
    print("kernel out", out.shape, out.dtype, float(np.abs(out).mean()))


# revision 5
# speedup vs baseline: 1.6883x; 1.6536x over previous
"""Multi-head attention (B=4, S=2048, D=1024, H=16) on 8 TRN2 NeuronCores.

Sharding: 2D grid batch(4) x head-group(2). Core c handles batch c//2 and
heads [ (c%2)*8 , (c%2)*8+8 ). Each core computes:
  - Q,K projections for its 8 heads in transposed layout q.T/k.T [512, 2048]
    (spilled to DRAM scratch, reloaded per 2-head group)
  - V projection in natural layout [2048, 512], stored ones-augmented per head
    ([128, 16, 8, 65] with col 64 = 1.0) so the AV matmul also produces the
    softmax denominators
  - attention per head: S.T = K Q^T (scores transposed), exp via ACT (scale
    fused), AV matmul accumulating C.T[hd,sq] + denom row, normalize via
    reciprocal + gpsimd partition-broadcast
  - output projection out.T = Wo_p.T^T @ ctx.T accumulated over head groups
Host side: shard/rearrange inputs, sum the two partial outputs per batch,
add bo, transpose back.

Matmul dtype: bfloat16 by default (KERNEL_DTYPE=f32r for float32r fallback).
PSUM accumulation is always fp32; biases and normalization are fp32.
"""
import os
import sys

sys.path.insert(0, "/opt/trn_rl_repo")

import types

# antenv.axon_hooks is missing from this image; install a shim so
# run_bass_kernel_spmd(trace=True) can reach the NTFF profiler.
if "antenv.axon_hooks" not in sys.modules:
    _mod = types.ModuleType("antenv.axon_hooks")
    _hook = [None]
    _mod.set_axon_ntff_profile_hook = lambda h: _hook.__setitem__(0, h)
    _mod.get_axon_ntff_profile_hook = lambda: _hook[0]
    sys.modules["antenv.axon_hooks"] = _mod
    import antenv

    antenv.axon_hooks = _mod
    try:
        from trn_agent_boot.trn_boot import _ntff_profile_via_ctypes

        _mod.set_axon_ntff_profile_hook(
            _ntff_profile_via_ctypes("/opt/axon/libaxon_pjrt.so")
        )
    except Exception:
        pass

import ml_dtypes
import numpy as np
import concourse.bass as bass
import concourse.mybir as mybir
import concourse.tile as tile
from concourse import bacc
from concourse import bass_utils

B, S, D, H = 4, 2048, 1024, 16
HD = D // H  # 64
SCALE = HD ** -0.5
NCORES = 8
HPC = 8          # heads per core
JP = HPC * HD    # 512 projected feature cols per core
G = 4            # 2-head groups per core
DC = D // 128    # 8 contraction chunks
NST = S // 128   # 16 s-tiles
F32 = mybir.dt.float32
F32R = mybir.dt.float32r

DTYPE_NAME = os.environ.get("KERNEL_DTYPE", "bf16")
if DTYPE_NAME == "bf16":
    MDT = mybir.dt.bfloat16
    NPDT = ml_dtypes.bfloat16
    SBP = 512    # s-block for projections
    PAD = True   # zero-pad QK contraction + AV stationary to full 128x128
else:
    MDT = F32R
    NPDT = np.float32
    SBP = 256
    PAD = False
NSB = S // SBP
VW = 128 if PAD else HD + 1  # v' head stride
AVP = 128 if PAD else HD + 1  # av psum partitions

_CACHE = {}


def _build():
    nc = bacc.Bacc("TRN2", target_bir_lowering=False, debug=False)

    xq = nc.dram_tensor("xq", [NSB, 128, DC, SBP], MDT, kind="ExternalInput").ap()
    xk = nc.dram_tensor("xk", [NSB, 128, DC, SBP], MDT, kind="ExternalInput").ap()
    xv = nc.dram_tensor("xv", [NSB, 128, DC, SBP], MDT, kind="ExternalInput").ap()
    wq = nc.dram_tensor("wq", [128, DC, JP], MDT, kind="ExternalInput").ap()
    wk = nc.dram_tensor("wk", [128, DC, JP], MDT, kind="ExternalInput").ap()
    wv = nc.dram_tensor("wv", [128, DC, JP], MDT, kind="ExternalInput").ap()
    wo = nc.dram_tensor("wo", [128, G, D], MDT, kind="ExternalInput").ap()
    bq = nc.dram_tensor("bq", [128, 4], F32, kind="ExternalInput").ap()
    bk = nc.dram_tensor("bk", [128, 4], F32, kind="ExternalInput").ap()
    bv = nc.dram_tensor("bv", [1, JP], F32, kind="ExternalInput").ap()
    out = nc.dram_tensor("out", [D, S], F32, kind="ExternalOutput").ap()

    with tile.TileContext(nc) as tc:
        with (
            tc.tile_pool(name="consts", bufs=1) as consts,
            tc.tile_pool(name="wts", bufs=3) as wts,
            tc.tile_pool(name="xs", bufs=2) as xs,
            tc.tile_pool(name="stage", bufs=3) as stage,
            tc.tile_pool(name="vpool", bufs=1) as vpool,
            tc.tile_pool(name="qk", bufs=2) as qkp,
            tc.tile_pool(name="ex", bufs=2) as exp_pool,
            tc.tile_pool(name="nrm", bufs=2) as nrm,
            tc.tile_pool(name="ctxp", bufs=4) as ctxp,
            tc.tile_pool(name="outst", bufs=3) as outst,
            tc.tile_pool(name="scr", bufs=1, space="DRAM") as scr,
        ):
            qsc = scr.tile([G, 128, S], MDT, tag="qsc")
            ksc = scr.tile([G, 128, S], MDT, tag="ksc")

            bq_sb = consts.tile([128, 4], F32, tag="bq")
            nc.sync.dma_start(out=bq_sb[:], in_=bq[:])
            bk_sb = consts.tile([128, 4], F32, tag="bk")
            nc.sync.dma_start(out=bk_sb[:], in_=bk[:])
            bv_row = consts.tile([1, JP], F32, tag="bvr")
            nc.sync.dma_start(out=bv_row[:], in_=bv[:])
            bv_bc = consts.tile([128, JP], F32, tag="bvb")
            nc.gpsimd.partition_broadcast(bv_bc[:], bv_row[:])

            # V' resident tile: [p, s_tile, head, VW]; col 64 = ones, cols
            # 65.. = zero pad (PAD mode keeps the PE array fully occupied and
            # FWL-eligible).
            v_sb = vpool.tile([128, NST, HPC, VW], MDT, tag="v")
            if PAD:
                nc.vector.memset(v_sb[:], 0.0)
            if MDT == F32R:
                nc.vector.memset(v_sb[:, :, :, HD:HD + 1].bitcast(F32), 1.0)
            else:
                nc.vector.memset(v_sb[:, :, :, HD:HD + 1], 1.0)

            # PAD mode: per-head zero-padded K tiles, persistent across groups.
            if PAD:
                k_all = vpool.tile([128, HPC, S], MDT, tag="kall")
                nc.vector.memset(k_all[:], 0.0)

            # ---------------- Q/K projections -> DRAM scratch ----------------
            with tc.tile_pool(name="pps", bufs=1, space="PSUM") as pps:
                for pname, xdram, wdram, bias_sb, scratch in (
                    ("q", xq, wq, bq_sb, qsc),
                    ("k", xk, wk, bk_sb, ksc),
                ):
                    w_sb = wts.tile([128, DC, JP], MDT, tag="w", name=f"w{pname}_sb")
                    nc.sync.dma_start(out=w_sb[:], in_=wdram[:])
                    for sb in range(NSB):
                        x_sb = xs.tile([128, DC, SBP], MDT, tag="x",
                                       name=f"x{pname}_{sb}")
                        nc.sync.dma_start(out=x_sb[:], in_=xdram[sb])
                        for jt in range(4):
                            ps_t = pps.tile([128, SBP], F32, tag="qk", bufs=4,
                                            name=f"ps{pname}_{sb}_{jt}")
                            for dc in range(DC):
                                nc.tensor.matmul(
                                    ps_t[:],
                                    w_sb[:, dc, jt * 128:(jt + 1) * 128],
                                    x_sb[:, dc, :],
                                    start=(dc == 0), stop=(dc == DC - 1),
                                )
                            st_t = stage.tile([128, SBP], MDT, tag="stg",
                                              name=f"st{pname}_{sb}_{jt}")
                            nc.vector.tensor_scalar_add(
                                out=st_t[:], in0=ps_t[:],
                                scalar1=bias_sb[:, jt:jt + 1],
                            )
                            nc.sync.dma_start(
                                out=scratch[jt, :, sb * SBP:(sb + 1) * SBP],
                                in_=st_t[:],
                            )

                # ---------------- V projection -> resident v_sb ----------------
                wv_sb = wts.tile([128, DC, JP], MDT, tag="w", name="wv_sb")
                nc.sync.dma_start(out=wv_sb[:], in_=wv[:])
                for sb in range(NSB):
                    xv_sb = xs.tile([128, DC, SBP], MDT, tag="x", name=f"xv_{sb}")
                    nc.sync.dma_start(out=xv_sb[:], in_=xv[sb])
                    for half in range(SBP // 128):
                        sti = sb * (SBP // 128) + half
                        ps_v = pps.tile([128, JP], F32, tag="v", bufs=2,
                                        name=f"psv_{sti}")
                        for dc in range(DC):
                            nc.tensor.matmul(
                                ps_v[:],
                                xv_sb[:, dc, half * 128:(half + 1) * 128],
                                wv_sb[:, dc, :],
                                start=(dc == 0), stop=(dc == DC - 1),
                            )
                        nc.vector.tensor_add(
                            out=v_sb[:, sti, :, 0:HD],
                            in0=ps_v.rearrange("p (h d) -> p h d", h=HPC),
                            in1=bv_bc.rearrange("p (h d) -> p h d", h=HPC),
                        )

            # ---------------- attention ----------------
            ctx_tiles = []
            with tc.tile_pool(name="aps", bufs=1, space="PSUM") as aps:
                for g in range(G):
                    q_sb = qkp.tile([128, S], MDT, tag="qg", name=f"qg_{g}")
                    nc.sync.dma_start(out=q_sb[:], in_=qsc[g])
                    if PAD:
                        nc.sync.dma_start(out=k_all[0:64, 2 * g, :],
                                          in_=ksc[g, 0:64, :])
                        nc.sync.dma_start(out=k_all[64:128, 2 * g + 1, :],
                                          in_=ksc[g, 64:128, :])
                    else:
                        k_sb = qkp.tile([128, S], MDT, tag="kg", name=f"kg_{g}")
                        nc.sync.dma_start(out=k_sb[:], in_=ksc[g])
                    ctx_t = ctxp.tile([128, S], MDT, tag="ctx", name=f"ctx_{g}")
                    ctx_tiles.append(ctx_t)
                    for hh in range(2):
                        p0 = hh * HD
                        hloc = 2 * g + hh
                        for sqp in range(2):
                            q0 = sqp * 1024
                            av = aps.tile([AVP, 2, 512], F32, tag="av", bufs=2,
                                          name=f"av_{g}_{hh}_{sqp}")
                            for sk in range(NST):
                                st = aps.tile([128, 2, 512], F32, tag="st", bufs=2,
                                              name=f"stt_{g}_{hh}_{sqp}_{sk}")
                                for i in range(2):
                                    if PAD:
                                        lhs_k = k_all[:, hloc, sk * 128:(sk + 1) * 128]
                                        rhs_q = q_sb[:, q0 + i * 512:q0 + (i + 1) * 512]
                                    else:
                                        lhs_k = k_sb[p0:p0 + HD, sk * 128:(sk + 1) * 128]
                                        rhs_q = q_sb[p0:p0 + HD,
                                                     q0 + i * 512:q0 + (i + 1) * 512]
                                    nc.tensor.matmul(
                                        st[:, i, :], lhs_k, rhs_q,
                                        start=True, stop=True,
                                    )
                                ex_t = exp_pool.tile([128, 2, 512], MDT, tag="ex",
                                                     name=f"ex_{g}_{hh}_{sqp}_{sk}")
                                nc.scalar.activation(
                                    out=ex_t[:], in_=st[:],
                                    func=mybir.ActivationFunctionType.Exp,
                                    scale=SCALE,
                                )
                                for i in range(2):
                                    nc.tensor.matmul(
                                        av[:, i, :],
                                        v_sb[:, sk, hloc, :],
                                        ex_t[:, i, :],
                                        start=(sk == 0), stop=(sk == NST - 1),
                                    )
                            for i in range(2):
                                dcp = nrm.tile([1, 512], F32, tag="dcp",
                                               name=f"dcp_{g}_{hh}_{sqp}_{i}")
                                nc.vector.tensor_copy(
                                    out=dcp[:], in_=av[HD:HD + 1, i, :])
                                recip = nrm.tile([1, 512], F32, tag="rc",
                                                 name=f"rc_{g}_{hh}_{sqp}_{i}")
                                nc.vector.reciprocal_approx_fast(
                                    recip[:], dcp[:])
                                bcast = nrm.tile([HD, 512], F32, tag="bc",
                                                 name=f"bc_{g}_{hh}_{sqp}_{i}")
                                nc.gpsimd.partition_broadcast(bcast[:], recip[:])
                                nc.vector.tensor_mul(
                                    out=ctx_t[p0:p0 + HD,
                                              q0 + i * 512:q0 + (i + 1) * 512],
                                    in0=av[0:HD, i, :],
                                    in1=bcast[:],
                                )

            # ---------------- output projection ----------------
            with tc.tile_pool(name="ops", bufs=1, space="PSUM") as ops:
                wo_sb = wts.tile([128, G, D], MDT, tag="w", name="wo_sb")
                nc.sync.dma_start(out=wo_sb[:], in_=wo[:])
                for et in range(D // 128):
                    for sb4 in range(4):
                        ps_o = ops.tile([128, 512], F32, tag="o", bufs=4,
                                        name=f"pso_{et}_{sb4}")
                        for g in range(G):
                            nc.tensor.matmul(
                                ps_o[:],
                                wo_sb[:, g, et * 128:(et + 1) * 128],
                                ctx_tiles[g][:, sb4 * 512:(sb4 + 1) * 512],
                                start=(g == 0), stop=(g == G - 1),
                            )
                        o_sb = outst.tile([128, 512], F32, tag="os",
                                          name=f"os_{et}_{sb4}")
                        nc.vector.tensor_copy(out=o_sb[:], in_=ps_o[:])
                        nc.sync.dma_start(
                            out=out[et * 128:(et + 1) * 128,
                                    sb4 * 512:(sb4 + 1) * 512],
                            in_=o_sb[:],
                        )
    nc.compile()
    return nc


def get_nc():
    if "nc" not in _CACHE:
        _CACHE["nc"] = _build()
    return _CACHE["nc"]


def _arrange_x(x):
    """[S, D] -> [NSB, 128, DC, SBP] with [sb, p, dc, s] = x[sb*SBP+s, dc*128+p]."""
    return np.ascontiguousarray(
        x.reshape(NSB, SBP, DC, 128).transpose(0, 3, 2, 1)
    ).astype(NPDT)


def _arrange_w(w_p):
    """W_p [JP, D] (rows for this core's heads) -> [128, DC, JP] lhsT layout."""
    # want [p, dc, j] = W_p.T[dc*128+p, j] = W_p[j, dc*128+p]
    return np.ascontiguousarray(
        w_p.T.reshape(DC, 128, JP).transpose(1, 0, 2)).astype(NPDT)


def _arrange_wo(wo_p):
    """Wo_p = Wo[:, cols] [D, JP] -> [128, G, D] with [p,g,e] = Wo_p[e, g*128+p]."""
    return np.ascontiguousarray(
        wo_p.T.reshape(G, 128, D).transpose(1, 0, 2)).astype(NPDT)


def prepare_in_maps(query, key, value, Wq, bq, Wk, bk, Wv, bv, Wo, bo):
    xs_arr = {}
    for b in range(B):
        xs_arr[("q", b)] = _arrange_x(np.asarray(query[b], np.float32))
        xs_arr[("k", b)] = _arrange_x(np.asarray(key[b], np.float32))
        xs_arr[("v", b)] = _arrange_x(np.asarray(value[b], np.float32))
    ws = {}
    for gidx in range(2):
        rows = slice(gidx * JP, (gidx + 1) * JP)
        ws[("wq", gidx)] = _arrange_w(np.asarray(Wq, np.float32)[rows])
        ws[("wk", gidx)] = _arrange_w(np.asarray(Wk, np.float32)[rows])
        ws[("wv", gidx)] = _arrange_w(np.asarray(Wv, np.float32)[rows])
        ws[("wo", gidx)] = _arrange_wo(np.asarray(Wo, np.float32)[:, rows])
        ws[("bq", gidx)] = np.ascontiguousarray(
            np.asarray(bq, np.float32)[rows].reshape(4, 128).T)
        ws[("bk", gidx)] = np.ascontiguousarray(
            np.asarray(bk, np.float32)[rows].reshape(4, 128).T)
        ws[("bv", gidx)] = np.asarray(bv, np.float32)[rows].reshape(1, JP).copy()
    in_maps = []
    for c in range(NCORES):
        b, gidx = c // 2, c % 2
        in_maps.append({
            "xq": xs_arr[("q", b)],
            "xk": xs_arr[("k", b)],
            "xv": xs_arr[("v", b)],
            "wq": ws[("wq", gidx)],
            "wk": ws[("wk", gidx)],
            "wv": ws[("wv", gidx)],
            "wo": ws[("wo", gidx)],
            "bq": ws[("bq", gidx)],
            "bk": ws[("bk", gidx)],
            "bv": ws[("bv", gidx)],
        })
    return in_maps


def run_hw(inputs, trace=False, trace_cores=None):
    nc = get_nc()
    in_maps = prepare_in_maps(**inputs)
    res = bass_utils.run_bass_kernel_spmd(
        nc, in_maps, core_ids=list(range(NCORES)),
        trace=trace, trace_cores=trace_cores,
    )
    bo = np.asarray(inputs["bo"], np.float32)
    out = np.empty((B, S, D), np.float32)
    for b in range(B):
        acc = res.results[2 * b]["out"] + res.results[2 * b + 1]["out"]
        out[b] = acc.T + bo
    return out, res


def kernel(**inputs):
    out, _ = run_hw(inputs, trace=False)
    return out


if __name__ == "__main__":
    rng = np.random.default_rng(0)
    ins = {
        "query": rng.standard_normal((B, S, D), np.float32),
        "key": rng.standard_normal((B, S, D), np.float32),
        "value": rng.standard_normal((B, S, D), np.float32),
        "Wq": (rng.standard_normal((D, D)) * D ** -0.5).astype(np.float32),
        "bq": np.zeros(D, np.float32),
        "Wk": (rng.standard_normal((D, D)) * D ** -0.5).astype(np.float32),
        "bk": np.zeros(D, np.float32),
        "Wv": (rng.standard_normal((D, D)) * D ** -0.5).astype(np.float32),
        "bv": np.zeros(D, np.float32),
        "Wo": (rng.standard_normal((D, D)) * D ** -0.5).astype(np.float32),
        "bo": np.zeros(D, np.float32),
    }
    out = kernel(**ins)
    print("kernel out", out.shape, out.dtype, float(np.abs(out).mean()))


# revision 7
# speedup vs baseline: 1.7391x; 1.0301x over previous
"""Multi-head attention (B=4, S=2048, D=1024, H=16) on 8 TRN2 NeuronCores.

Sharding: 2D grid batch(4) x head-group(2). Core c handles batch c//2 and
heads [ (c%2)*8 , (c%2)*8+8 ). Each core computes:
  - Q,K projections for its 8 heads in transposed layout q.T/k.T [512, 2048]
    (spilled to DRAM scratch, reloaded per 2-head group)
  - V projection in natural layout [2048, 512], stored ones-augmented per head
    ([128, 16, 8, 65] with col 64 = 1.0) so the AV matmul also produces the
    softmax denominators
  - attention per head: S.T = K Q^T (scores transposed), exp via ACT (scale
    fused), AV matmul accumulating C.T[hd,sq] + denom row, normalize via
    reciprocal + gpsimd partition-broadcast
  - output projection out.T = Wo_p.T^T @ ctx.T accumulated over head groups
Host side: shard/rearrange inputs, sum the two partial outputs per batch,
add bo, transpose back.

Matmul dtype: bfloat16 by default (KERNEL_DTYPE=f32r for float32r fallback).
PSUM accumulation is always fp32; biases and normalization are fp32.
"""
import os
import sys

sys.path.insert(0, "/opt/trn_rl_repo")

import types

# antenv.axon_hooks is missing from this image; install a shim so
# run_bass_kernel_spmd(trace=True) can reach the NTFF profiler.
if "antenv.axon_hooks" not in sys.modules:
    _mod = types.ModuleType("antenv.axon_hooks")
    _hook = [None]
    _mod.set_axon_ntff_profile_hook = lambda h: _hook.__setitem__(0, h)
    _mod.get_axon_ntff_profile_hook = lambda: _hook[0]
    sys.modules["antenv.axon_hooks"] = _mod
    import antenv

    antenv.axon_hooks = _mod
    try:
        from trn_agent_boot.trn_boot import _ntff_profile_via_ctypes

        _mod.set_axon_ntff_profile_hook(
            _ntff_profile_via_ctypes("/opt/axon/libaxon_pjrt.so")
        )
    except Exception:
        pass

import ml_dtypes
import numpy as np
import concourse.bass as bass
import concourse.mybir as mybir
import concourse.tile as tile
from concourse import bacc
from concourse import bass_utils

B, S, D, H = 4, 2048, 1024, 16
HD = D // H  # 64
SCALE = HD ** -0.5
NCORES = 8
HPC = 8          # heads per core
JP = HPC * HD    # 512 projected feature cols per core
G = 4            # 2-head groups per core
DC = D // 128    # 8 contraction chunks
NST = S // 128   # 16 s-tiles
F32 = mybir.dt.float32
F32R = mybir.dt.float32r

DTYPE_NAME = os.environ.get("KERNEL_DTYPE", "bf16")
if DTYPE_NAME == "bf16":
    MDT = mybir.dt.bfloat16
    NPDT = ml_dtypes.bfloat16
    SBP = 512    # s-block for projections
    PAD = True   # zero-pad QK contraction + AV stationary to full 128x128
else:
    MDT = F32R
    NPDT = np.float32
    SBP = 256
    PAD = False
NSB = S // SBP
VW = 128 if PAD else HD + 1  # v' head stride
AVP = 128 if PAD else HD + 1  # av psum partitions

_CACHE = {}


def _build():
    nc = bacc.Bacc("TRN2", target_bir_lowering=False, debug=False)

    xq = nc.dram_tensor("xq", [NSB, 128, DC, SBP], MDT, kind="ExternalInput").ap()
    xk = nc.dram_tensor("xk", [NSB, 128, DC, SBP], MDT, kind="ExternalInput").ap()
    xv = nc.dram_tensor("xv", [NSB, 128, DC, SBP], MDT, kind="ExternalInput").ap()
    wq = nc.dram_tensor("wq", [128, DC, JP], MDT, kind="ExternalInput").ap()
    wk = nc.dram_tensor("wk", [128, DC, JP], MDT, kind="ExternalInput").ap()
    wv = nc.dram_tensor("wv", [128, DC, JP], MDT, kind="ExternalInput").ap()
    wo = nc.dram_tensor("wo", [128, G, D], MDT, kind="ExternalInput").ap()
    bq = nc.dram_tensor("bq", [128, 4], F32, kind="ExternalInput").ap()
    bk = nc.dram_tensor("bk", [128, 4], F32, kind="ExternalInput").ap()
    bv = nc.dram_tensor("bv", [1, JP], F32, kind="ExternalInput").ap()
    out = nc.dram_tensor("out", [D, S], F32, kind="ExternalOutput").ap()

    with tile.TileContext(nc) as tc:
        with (
            tc.tile_pool(name="consts", bufs=1) as consts,
            tc.tile_pool(name="wts", bufs=3) as wts,
            tc.tile_pool(name="xs", bufs=2) as xs,
            tc.tile_pool(name="stage", bufs=3) as stage,
            tc.tile_pool(name="vpool", bufs=1) as vpool,
            tc.tile_pool(name="qk", bufs=2) as qkp,
            tc.tile_pool(name="ex", bufs=2) as exp_pool,
            tc.tile_pool(name="nrm", bufs=2) as nrm,
            tc.tile_pool(name="ctxp", bufs=4) as ctxp,
            tc.tile_pool(name="outst", bufs=3) as outst,
            tc.tile_pool(name="scr", bufs=1, space="DRAM") as scr,
        ):
            qsc = scr.tile([G, 128, S], MDT, tag="qsc")
            ksc = scr.tile([G, 128, S], MDT, tag="ksc")

            bq_sb = consts.tile([128, 4], F32, tag="bq")
            nc.sync.dma_start(out=bq_sb[:], in_=bq[:])
            bk_sb = consts.tile([128, 4], F32, tag="bk")
            nc.sync.dma_start(out=bk_sb[:], in_=bk[:])
            bv_row = consts.tile([1, JP], F32, tag="bvr")
            nc.sync.dma_start(out=bv_row[:], in_=bv[:])
            bv_bc = consts.tile([128, JP], F32, tag="bvb")
            nc.gpsimd.partition_broadcast(bv_bc[:], bv_row[:])

            # V' resident tile: [p, s_tile, head, VW]; col 64 = ones, cols
            # 65.. = zero pad (PAD mode keeps the PE array fully occupied and
            # FWL-eligible).
            v_sb = vpool.tile([128, NST, HPC, VW], MDT, tag="v")
            if PAD:
                nc.vector.memset(v_sb[:], 0.0)
            if MDT == F32R:
                nc.vector.memset(v_sb[:, :, :, HD:HD + 1].bitcast(F32), 1.0)
            else:
                nc.vector.memset(v_sb[:, :, :, HD:HD + 1], 1.0)

            # PAD mode: per-head zero-padded K tiles, persistent across groups.
            if PAD:
                k_all = vpool.tile([128, HPC, S], MDT, tag="kall")
                nc.vector.memset(k_all[:], 0.0)

            # ---------------- Q/K projections -> DRAM scratch ----------------
            with tc.tile_pool(name="pps", bufs=1, space="PSUM") as pps:
                for pname, xdram, wdram, bias_sb, scratch in (
                    ("q", xq, wq, bq_sb, qsc),
                    ("k", xk, wk, bk_sb, ksc),
                ):
                    w_sb = wts.tile([128, DC, JP], MDT, tag="w", name=f"w{pname}_sb")
                    for dh in range(2):
                        nc.sync.dma_start(out=w_sb[:, dh * 4:(dh + 1) * 4, :],
                                          in_=wdram[:, dh * 4:(dh + 1) * 4, :])
                    for sb in range(NSB):
                        x_sb = xs.tile([128, DC, SBP], MDT, tag="x",
                                       name=f"x{pname}_{sb}", bufs=3)
                        for dh in range(4):
                            nc.sync.dma_start(
                                out=x_sb[:, dh * 2:(dh + 1) * 2, :],
                                in_=xdram[sb, :, dh * 2:(dh + 1) * 2, :])
                        for jt in range(4):
                            ps_t = pps.tile([128, SBP], F32, tag="qk", bufs=4,
                                            name=f"ps{pname}_{sb}_{jt}")
                            for dc in range(DC):
                                nc.tensor.matmul(
                                    ps_t[:],
                                    w_sb[:, dc, jt * 128:(jt + 1) * 128],
                                    x_sb[:, dc, :],
                                    start=(dc == 0), stop=(dc == DC - 1),
                                )
                            st_t = stage.tile([128, SBP], MDT, tag="stg",
                                              name=f"st{pname}_{sb}_{jt}")
                            nc.vector.tensor_scalar_add(
                                out=st_t[:], in0=ps_t[:],
                                scalar1=bias_sb[:, jt:jt + 1],
                            )
                            nc.sync.dma_start(
                                out=scratch[jt, :, sb * SBP:(sb + 1) * SBP],
                                in_=st_t[:],
                            )

                # ---------------- V projection -> resident v_sb ----------------
                wv_sb = wts.tile([128, DC, JP], MDT, tag="w", name="wv_sb")
                nc.sync.dma_start(out=wv_sb[:], in_=wv[:])
                for sb in range(NSB):
                    xv_sb = xs.tile([128, DC, SBP], MDT, tag="x", name=f"xv_{sb}",
                                    bufs=3)
                    for dh in range(4):
                        nc.sync.dma_start(
                            out=xv_sb[:, dh * 2:(dh + 1) * 2, :],
                            in_=xv[sb, :, dh * 2:(dh + 1) * 2, :])
                    for half in range(SBP // 128):
                        sti = sb * (SBP // 128) + half
                        ps_v = pps.tile([128, JP], F32, tag="v", bufs=2,
                                        name=f"psv_{sti}")
                        for dc in range(DC):
                            nc.tensor.matmul(
                                ps_v[:],
                                xv_sb[:, dc, half * 128:(half + 1) * 128],
                                wv_sb[:, dc, :],
                                start=(dc == 0), stop=(dc == DC - 1),
                            )
                        nc.vector.tensor_add(
                            out=v_sb[:, sti, :, 0:HD],
                            in0=ps_v.rearrange("p (h d) -> p h d", h=HPC),
                            in1=bv_bc.rearrange("p (h d) -> p h d", h=HPC),
                        )

            # ---------------- attention ----------------
            ctx_tiles = []
            with tc.tile_pool(name="aps", bufs=1, space="PSUM") as aps:
                for g in range(G):
                    q_sb = qkp.tile([128, S], MDT, tag="qg", name=f"qg_{g}")
                    nc.sync.dma_start(out=q_sb[:], in_=qsc[g])
                    if PAD:
                        nc.sync.dma_start(out=k_all[0:64, 2 * g, :],
                                          in_=ksc[g, 0:64, :])
                        nc.sync.dma_start(out=k_all[64:128, 2 * g + 1, :],
                                          in_=ksc[g, 64:128, :])
                    else:
                        k_sb = qkp.tile([128, S], MDT, tag="kg", name=f"kg_{g}")
                        nc.sync.dma_start(out=k_sb[:], in_=ksc[g])
                    ctx_t = ctxp.tile([128, S], MDT, tag="ctx", name=f"ctx_{g}")
                    ctx_tiles.append(ctx_t)
                    for hh in range(2):
                        p0 = hh * HD
                        hloc = 2 * g + hh
                        for sqp in range(2):
                            q0 = sqp * 1024
                            av = aps.tile([AVP, 2, 512], F32, tag="av", bufs=2,
                                          name=f"av_{g}_{hh}_{sqp}")
                            for sk in range(NST):
                                st = aps.tile([128, 2, 512], F32, tag="st", bufs=2,
                                              name=f"stt_{g}_{hh}_{sqp}_{sk}")
                                for i in range(2):
                                    if PAD:
                                        lhs_k = k_all[:, hloc, sk * 128:(sk + 1) * 128]
                                        rhs_q = q_sb[:, q0 + i * 512:q0 + (i + 1) * 512]
                                    else:
                                        lhs_k = k_sb[p0:p0 + HD, sk * 128:(sk + 1) * 128]
                                        rhs_q = q_sb[p0:p0 + HD,
                                                     q0 + i * 512:q0 + (i + 1) * 512]
                                    nc.tensor.matmul(
                                        st[:, i, :], lhs_k, rhs_q,
                                        start=True, stop=True,
                                    )
                                ex_t = exp_pool.tile([128, 2, 512], MDT, tag="ex",
                                                     name=f"ex_{g}_{hh}_{sqp}_{sk}")
                                nc.scalar.activation(
                                    out=ex_t[:], in_=st[:],
                                    func=mybir.ActivationFunctionType.Exp,
                                    scale=SCALE,
                                )
                                for i in range(2):
                                    nc.tensor.matmul(
                                        av[:, i, :],
                                        v_sb[:, sk, hloc, :],
                                        ex_t[:, i, :],
                                        start=(sk == 0), stop=(sk == NST - 1),
                                    )
                            for i in range(2):
                                dcp = nrm.tile([1, 512], F32, tag="dcp",
                                               name=f"dcp_{g}_{hh}_{sqp}_{i}")
                                nc.vector.tensor_copy(
                                    out=dcp[:], in_=av[HD:HD + 1, i, :])
                                recip = nrm.tile([1, 512], F32, tag="rc",
                                                 name=f"rc_{g}_{hh}_{sqp}_{i}")
                                nc.vector.reciprocal_approx_fast(
                                    recip[:], dcp[:])
                                bcast = nrm.tile([HD, 512], F32, tag="bc",
                                                 name=f"bc_{g}_{hh}_{sqp}_{i}")
                                nc.gpsimd.partition_broadcast(bcast[:], recip[:])
                                nc.vector.tensor_mul(
                                    out=ctx_t[p0:p0 + HD,
                                              q0 + i * 512:q0 + (i + 1) * 512],
                                    in0=av[0:HD, i, :],
                                    in1=bcast[:],
                                )

            # ---------------- output projection ----------------
            with tc.tile_pool(name="ops", bufs=1, space="PSUM") as ops:
                wo_sb = wts.tile([128, G, D], MDT, tag="w", name="wo_sb")
                for dh in range(2):
                    nc.sync.dma_start(out=wo_sb[:, dh * 2:(dh + 1) * 2, :],
                                      in_=wo[:, dh * 2:(dh + 1) * 2, :])
                for et in range(D // 128):
                    for sb4 in range(4):
                        ps_o = ops.tile([128, 512], F32, tag="o", bufs=6,
                                        name=f"pso_{et}_{sb4}")
                        for g in range(G):
                            nc.tensor.matmul(
                                ps_o[:],
                                wo_sb[:, g, et * 128:(et + 1) * 128],
                                ctx_tiles[g][:, sb4 * 512:(sb4 + 1) * 512],
                                start=(g == 0), stop=(g == G - 1),
                            )
                        o_sb = outst.tile([128, 512], F32, tag="os", bufs=4,
                                          name=f"os_{et}_{sb4}")
                        nc.vector.tensor_copy(out=o_sb[:], in_=ps_o[:])
                        nc.sync.dma_start(
                            out=out[et * 128:(et + 1) * 128,
                                    sb4 * 512:(sb4 + 1) * 512],
                            in_=o_sb[:],
                        )
    nc.compile()
    return nc


def get_nc():
    if "nc" not in _CACHE:
        _CACHE["nc"] = _build()
    return _CACHE["nc"]


def _arrange_x(x):
    """[S, D] -> [NSB, 128, DC, SBP] with [sb, p, dc, s] = x[sb*SBP+s, dc*128+p]."""
    return np.ascontiguousarray(
        x.reshape(NSB, SBP, DC, 128).transpose(0, 3, 2, 1)
    ).astype(NPDT)


def _arrange_w(w_p):
    """W_p [JP, D] (rows for this core's heads) -> [128, DC, JP] lhsT layout."""
    # want [p, dc, j] = W_p.T[dc*128+p, j] = W_p[j, dc*128+p]
    return np.ascontiguousarray(
        w_p.T.reshape(DC, 128, JP).transpose(1, 0, 2)).astype(NPDT)


def _arrange_wo(wo_p):
    """Wo_p = Wo[:, cols] [D, JP] -> [128, G, D] with [p,g,e] = Wo_p[e, g*128+p]."""
    return np.ascontiguousarray(
        wo_p.T.reshape(G, 128, D).transpose(1, 0, 2)).astype(NPDT)


def prepare_in_maps(query, key, value, Wq, bq, Wk, bk, Wv, bv, Wo, bo):
    xs_arr = {}
    for b in range(B):
        xs_arr[("q", b)] = _arrange_x(np.asarray(query[b], np.float32))
        xs_arr[("k", b)] = _arrange_x(np.asarray(key[b], np.float32))
        xs_arr[("v", b)] = _arrange_x(np.asarray(value[b], np.float32))
    ws = {}
    for gidx in range(2):
        rows = slice(gidx * JP, (gidx + 1) * JP)
        ws[("wq", gidx)] = _arrange_w(np.asarray(Wq, np.float32)[rows])
        ws[("wk", gidx)] = _arrange_w(np.asarray(Wk, np.float32)[rows])
        ws[("wv", gidx)] = _arrange_w(np.asarray(Wv, np.float32)[rows])
        ws[("wo", gidx)] = _arrange_wo(np.asarray(Wo, np.float32)[:, rows])
        ws[("bq", gidx)] = np.ascontiguousarray(
            np.asarray(bq, np.float32)[rows].reshape(4, 128).T)
        ws[("bk", gidx)] = np.ascontiguousarray(
            np.asarray(bk, np.float32)[rows].reshape(4, 128).T)
        ws[("bv", gidx)] = np.asarray(bv, np.float32)[rows].reshape(1, JP).copy()
    in_maps = []
    for c in range(NCORES):
        b, gidx = c // 2, c % 2
        in_maps.append({
            "xq": xs_arr[("q", b)],
            "xk": xs_arr[("k", b)],
            "xv": xs_arr[("v", b)],
            "wq": ws[("wq", gidx)],
            "wk": ws[("wk", gidx)],
            "wv": ws[("wv", gidx)],
            "wo": ws[("wo", gidx)],
            "bq": ws[("bq", gidx)],
            "bk": ws[("bk", gidx)],
            "bv": ws[("bv", gidx)],
        })
    return in_maps


def run_hw(inputs, trace=False, trace_cores=None):
    nc = get_nc()
    in_maps = prepare_in_maps(**inputs)
    res = bass_utils.run_bass_kernel_spmd(
        nc, in_maps, core_ids=list(range(NCORES)),
        trace=trace, trace_cores=trace_cores,
    )
    bo = np.asarray(inputs["bo"], np.float32)
    out = np.empty((B, S, D), np.float32)
    for b in range(B):
        acc = res.results[2 * b]["out"] + res.results[2 * b + 1]["out"]
        out[b] = acc.T + bo
    return out, res


def kernel(**inputs):
    out, _ = run_hw(inputs, trace=False)
    return out


if __name__ == "__main__":
    rng = np.random.default_rng(0)
    ins = {
        "query": rng.standard_normal((B, S, D), np.float32),
        "key": rng.standard_normal((B, S, D), np.float32),
        "value": rng.standard_normal((B, S, D), np.float32),
        "Wq": (rng.standard_normal((D, D)) * D ** -0.5).astype(np.float32),
        "bq": np.zeros(D, np.float32),
        "Wk": (rng.standard_normal((D, D)) * D ** -0.5).astype(np.float32),
        "bk": np.zeros(D, np.float32),
        "Wv": (rng.standard_normal((D, D)) * D ** -0.5).astype(np.float32),
        "bv": np.zeros(D, np.float32),
        "Wo": (rng.standard_normal((D, D)) * D ** -0.5).astype(np.float32),
        "bo": np.zeros(D, np.float32),
    }
    out = kernel(**ins)
    print("kernel out", out.shape, out.dtype, float(np.abs(out).mean()))
